# revision 3
# baseline (speedup 1.0000x reference)
"""NeighborAttention (B=4, N=4096, K=32, C=128, H=4) on 8 Trainium2 cores.

Data-parallel: the flattened (B*N) node axis is sharded across 8 cores;
the four small weight matrices are replicated. Inside each core everything
is channel-major ([row 4d+h, node-major free]):

  ET = (E*mask)^T            -> K,V of masked neighbors are exactly 0
  KT = WK' @ ET, VT = WV' @ ET, QT = (WQ'/sqrt(d)) @ XT        (PE)
  prod = KT * broadcast_j(QT)                                  (DVE)
  s_rep = Hrep @ prod        head-summed scores, replicated over d (PE)
  e = exp(s_rep)             no max-subtraction needed: |s| is small, and
                             softmax is shift-invariant               (ACT)
  z = sum_j e - (K - cnt[n]) masked j contribute exp(0)=1; host sends the
                             per-node count correction               (DVE)
  uv = e * VT;  umax = max_j uv;  usum = sum_j uv                    (DVE)
  out = (WO_mean+WO_sum)' @ (usum/z) + WO_max' @ (umax/z)            (PE)

attn sums to exactly 1, so aggr_mean == aggr_sum (within 1e-8) and the
mean/sum W_O blocks fold together on the host.
"""
import numpy as np
import concourse.bass as bass
import concourse.bacc as bacc
import concourse.mybir as mybir
from concourse import tile
from concourse.bass_utils import run_bass_kernel_spmd

F32 = mybir.dt.float32
AX = mybir.AxisListType.X
ALU = mybir.AluOpType

K = 32
C = 128
H = 4
D = 32
NCORES = 8

SUB_N = 16
SUB_COLS = SUB_N * K
CHUNK_N = 128
CHUNK_COLS = CHUNK_N * K

_NC_CACHE = {}


def _build_nc(nloc):
    assert nloc % CHUNK_N == 0
    if nloc in _NC_CACHE:
        return _NC_CACHE[nloc]
    nchunks = nloc // CHUNK_N
    nsub = CHUNK_COLS // SUB_COLS

    nc = bacc.Bacc()
    et = nc.dram_tensor("et", [C, nloc * K], F32, kind="ExternalInput")
    xt = nc.dram_tensor("xt", [C, nloc], F32, kind="ExternalInput")
    wqt = nc.dram_tensor("wqt", [C, C], F32, kind="ExternalInput")
    wkt = nc.dram_tensor("wkt", [C, C], F32, kind="ExternalInput")
    wvt = nc.dram_tensor("wvt", [C, C], F32, kind="ExternalInput")
    hrep = nc.dram_tensor("hrep", [C, C], F32, kind="ExternalInput")
    wost = nc.dram_tensor("wost", [C, C], F32, kind="ExternalInput")
    wo3t = nc.dram_tensor("wo3t", [C, C], F32, kind="ExternalInput")
    mcorr = nc.dram_tensor("mcorr", [C, nloc], F32, kind="ExternalInput")
    out = nc.dram_tensor("out", [C, nloc], F32, kind="ExternalOutput")

    with tile.TileContext(nc) as tc:
        with tc.tile_pool(name="wts", bufs=1) as wpool, \
             tc.tile_pool(name="xin", bufs=1) as xpool, \
             tc.tile_pool(name="etp", bufs=3) as etpool, \
             tc.tile_pool(name="work", bufs=5) as work, \
             tc.tile_pool(name="acc", bufs=1) as accp, \
             tc.tile_pool(name="epi", bufs=1) as epip, \
             tc.tile_pool(name="outp", bufs=1) as outp, \
             tc.tile_pool(name="pkv", bufs=5, space="PSUM") as pkv, \
             tc.tile_pool(name="psc", bufs=2, space="PSUM") as psc, \
             tc.tile_pool(name="psmall", bufs=1, space="PSUM") as psmall:

            w_q = wpool.tile([C, C], F32, tag="wq")
            w_k = wpool.tile([C, C], F32, tag="wk")
            w_v = wpool.tile([C, C], F32, tag="wv")
            w_h = wpool.tile([C, C], F32, tag="wh")
            w_os = wpool.tile([C, C], F32, tag="wos")
            w_o3 = wpool.tile([C, C], F32, tag="wo3")
            nc.sync.dma_start(w_q[:], wqt[:])
            nc.sync.dma_start(w_k[:], wkt[:])
            nc.sync.dma_start(w_v[:], wvt[:])
            nc.sync.dma_start(w_h[:], hrep[:])
            nc.sync.dma_start(w_os[:], wost[:])
            nc.sync.dma_start(w_o3[:], wo3t[:])

            xt_sb = xpool.tile([C, nloc], F32, tag="xt")
            nc.sync.dma_start(xt_sb[:], xt[:])
            mc_sb = xpool.tile([C, nloc], F32, tag="mc")
            nc.sync.dma_start(mc_sb[:], mcorr[:])

            out_sb = outp.tile([C, nloc], F32, tag="osb")

            umax_c = accp.tile([C, nloc], F32, tag="umax")
            usum_c = accp.tile([C, nloc], F32, tag="usum")
            z_c = accp.tile([C, nloc], F32, tag="zc")

            for ch in range(nchunks):
                n0 = ch * CHUNK_N
                c0 = ch * CHUNK_COLS

                et_sb = etpool.tile([C, CHUNK_COLS], F32, tag="et")
                nc.sync.dma_start(et_sb[:], et[:, c0:c0 + CHUNK_COLS])

                q_ps = psmall.tile([C, CHUNK_N], F32, tag="qo")
                nc.tensor.matmul(q_ps[:], w_q[:], xt_sb[:, n0:n0 + CHUNK_N],
                                 start=True, stop=True)
                q_sb = work.tile([C, CHUNK_N], F32, tag="qsb")
                nc.vector.tensor_copy(q_sb[:], q_ps[:])

                for s in range(nsub):
                    sc0 = s * SUB_COLS
                    snl = s * SUB_N
                    sn0 = n0 + snl
                    esl = et_sb[:, sc0:sc0 + SUB_COLS]

                    kt_ps = pkv.tile([C, SUB_COLS], F32, tag="kv")
                    nc.tensor.matmul(kt_ps[:], w_k[:], esl, start=True, stop=True)
                    vt_ps = pkv.tile([C, SUB_COLS], F32, tag="kv")
                    nc.tensor.matmul(vt_ps[:], w_v[:], esl, start=True, stop=True)

                    qb = q_sb[:, snl:snl + SUB_N].unsqueeze(2).broadcast_to(
                        (C, SUB_N, K))
                    prod = work.tile([C, SUB_COLS], F32, tag="prod")
                    nc.vector.tensor_mul(
                        prod[:].rearrange("p (n j) -> p n j", j=K),
                        kt_ps[:].rearrange("p (n j) -> p n j", j=K),
                        qb)

                    s_ps = psc.tile([C, SUB_COLS], F32, tag="srep")
                    nc.tensor.matmul(s_ps[:], w_h[:], prod[:],
                                     start=True, stop=True)

                    erep = work.tile([C, SUB_COLS], F32, tag="erep")
                    nc.scalar.activation(erep[:], s_ps[:],
                                         mybir.ActivationFunctionType.Exp)

                    uv = work.tile([C, SUB_COLS], F32, tag="uv")
                    nc.vector.tensor_mul(uv[:], erep[:], vt_ps[:])

                    uv_v = uv[:].rearrange("p (n j) -> p n j", j=K)
                    e_v = erep[:].rearrange("p (n j) -> p n j", j=K)
                    nc.vector.tensor_reduce(
                        umax_c[:, sn0:sn0 + SUB_N], uv_v, axis=AX, op=ALU.max)
                    nc.vector.tensor_reduce(
                        usum_c[:, sn0:sn0 + SUB_N], uv_v, axis=AX, op=ALU.add)
                    nc.vector.tensor_reduce(
                        z_c[:, sn0:sn0 + SUB_N], e_v, axis=AX, op=ALU.add)

            zcor = epip.tile([C, nloc], F32, tag="zcor")
            nc.vector.tensor_sub(zcor[:], z_c[:], mc_sb[:])
            # fully-masked nodes: umax/usum rows are exactly 0, so any
            # finite 1/z gives the correct 0 output — just avoid inf*0.
            nc.vector.tensor_scalar_max(zcor[:], zcor[:], 1e-20)
            rz = epip.tile([C, nloc], F32, tag="rz")
            nc.vector.reciprocal(rz[:], zcor[:])

            wsn = epip.tile([C, nloc], F32, tag="wsn")
            nc.vector.tensor_mul(wsn[:], usum_c[:], rz[:])
            mxn = epip.tile([C, nloc], F32, tag="mxn")
            nc.vector.tensor_mul(mxn[:], umax_c[:], rz[:])

            ob = min(512, nloc)
            for b0 in range(0, nloc, ob):
                o_ps = psmall.tile([C, ob], F32, tag="qo")
                nc.tensor.matmul(o_ps[:], w_os[:], wsn[:, b0:b0 + ob],
                                 start=True, stop=False)
                nc.tensor.matmul(o_ps[:], w_o3[:], mxn[:, b0:b0 + ob],
                                 start=False, stop=True)
                nc.scalar.copy(out_sb[:, b0:b0 + ob], o_ps[:])

            nc.sync.dma_start(out[:], out_sb[:])

    nc.compile()
    _NC_CACHE[nloc] = nc
    return nc


def _perm_dh(w):
    """[(h*32+d), cin] -> [cin, (4d+h)]"""
    wt = np.asarray(w).reshape(H, D, -1)
    return np.ascontiguousarray(np.transpose(wt, (2, 1, 0)).reshape(-1, H * D))


def build_nc(nloc):
    return _build_nc(nloc)


def prep_inputs(h_X, h_E, mask_attn, W_Q, W_K, W_V, W_O):
    h_X = np.asarray(h_X, dtype=np.float32)
    h_E = np.asarray(h_E, dtype=np.float32)
    mask_attn = np.asarray(mask_attn)
    W_Q = np.asarray(W_Q, dtype=np.float32)
    W_K = np.asarray(W_K, dtype=np.float32)
    W_V = np.asarray(W_V, dtype=np.float32)
    W_O = np.asarray(W_O, dtype=np.float32)

    B, N, Kn, Cin = h_E.shape
    BN = B * N
    nloc = BN // NCORES

    maskf = mask_attn.astype(np.float32)
    e_m = (h_E * maskf[..., None]).reshape(BN, Kn, Cin)
    xf = h_X.reshape(BN, -1)
    cnt = maskf.reshape(BN, Kn).sum(axis=1)

    wqt = _perm_dh(W_Q / np.sqrt(D))
    wkt = _perm_dh(W_K)
    wvt = _perm_dh(W_V)

    idx = np.arange(C)
    hh = idx % H
    hrep = (hh[:, None] == hh[None, :]).astype(np.float32)

    wos = W_O[:, :C] + W_O[:, C:2 * C]
    wo3 = W_O[:, 2 * C:]
    wost = np.ascontiguousarray(
        wos.T.reshape(H, D, C).transpose(1, 0, 2).reshape(C, C))
    wo3t = np.ascontiguousarray(
        wo3.T.reshape(H, D, C).transpose(1, 0, 2).reshape(C, C))

    in_maps = []
    for i in range(NCORES):
        sl = slice(i * nloc, (i + 1) * nloc)
        etc = np.ascontiguousarray(e_m[sl].reshape(nloc * Kn, Cin).T)
        xtc = np.ascontiguousarray(xf[sl].T)
        mc = np.ascontiguousarray(
            np.broadcast_to(Kn - cnt[sl], (C, nloc)).astype(np.float32))
        in_maps.append({
            "et": etc, "xt": xtc,
            "wqt": wqt, "wkt": wkt, "wvt": wvt, "hrep": hrep,
            "wost": wost, "wo3t": wo3t, "mcorr": mc,
        })
    return in_maps, nloc


def assemble_output(results, B, N):
    BN = B * N
    nloc = BN // NCORES
    outf = np.empty((BN, C), np.float32)
    for i, r in enumerate(results):
        outf[i * nloc:(i + 1) * nloc] = r["out"].T
    return outf.reshape(B, N, C)


def kernel(h_X, h_E, mask_attn, W_Q, W_K, W_V, W_O):
    B, N = np.asarray(h_X).shape[:2]
    in_maps, nloc = prep_inputs(h_X, h_E, mask_attn, W_Q, W_K, W_V, W_O)
    nc = _build_nc(nloc)
    res = run_bass_kernel_spmd(nc, in_maps, core_ids=list(range(NCORES)))
    return assemble_output(res.results, B, N)



# revision 11
# speedup vs baseline: 2.6525x; 2.6525x over previous
"""NeighborAttention (B=4, N=4096, K=32, C=128, H=4) on 8 Trainium2 cores.

Data-parallel over the flattened (B*N) node axis; weights replicated.
Per-core layout is channel-major: partition c = 4d+h, free axis j-major
per chunk: col = j*CH + n.

Key ideas vs the fp32 baseline:
- Neighbor compaction: the attention mask is ~50% dense, so each node's
  active neighbors are packed (host-side gather) into the smallest
  bucket Kb >= cnt, Kb in {12,16,20,24,28,32}. Padded slots are all-zero
  E columns -> k=v=0, s=0, e=exp(0)=1; z is corrected by mcorr = Kb-cnt
  and uv=0 pads reproduce the reference's "masked entries are exactly 0
  in the max" semantics.
- bf16 everywhere on the big tensors: PE matmuls run 1 cyc/row (vs 4 for
  fp32) and DVE elementwise ops hit the 2x_1p mode.
- Engine balance: PE does K/V/score projections plus the j-reductions
  that are sums (z and usum as PSUM-accumulated identity matmuls); ACT
  does exp + KT evacuation; GpSimd does VT evacuation; DVE does the two
  elementwise muls, the pairwise max-tree, and the tiny epilogue.
- Divide-late softmax: usum/umax are divided by z at [C, CH] size.
  attn sums to 1 so the mean/sum W_O blocks fold on the host.
"""
import numpy as np
import concourse.bass as bass
import concourse.bacc as bacc
import concourse.mybir as mybir
from concourse import tile
from concourse.bass_utils import run_bass_kernel_spmd

F32 = mybir.dt.float32
BF16 = mybir.dt.bfloat16
NPBF16 = mybir.dt.np(mybir.dt.bfloat16)
ALU = mybir.AluOpType
ACTF = mybir.ActivationFunctionType

K = 32
C = 128
H = 4
D = 32
NCORES = 8

BUCKET_KS = [12, 16, 20, 24, 28, 32]

_NC_CACHE = {}


def _pieces(Kb, CH):
    """Split the j axis into groups so each piece is <= 512 cols."""
    jpp = max(1, 512 // CH)
    out = []
    j = 0
    while j < Kb:
        out.append((j, min(j + jpp, Kb)))
        j += jpp
    return out


def _build_nc(spec):
    """spec: tuple of (Kb, CH, nchunks) per active bucket."""
    if spec in _NC_CACHE:
        return _NC_CACHE[spec]
    nloc = sum(ch * nch for (_, ch, nch) in spec)
    cols = sum(kb * ch * nch for (kb, ch, nch) in spec)
    maxc = max(kb * ch for (kb, ch, _) in spec)
    maxh = max(((kb // 2) + 1) * ch for (kb, ch, _) in spec)

    nc = bacc.Bacc()
    et = nc.dram_tensor("et", [C, cols], BF16, kind="ExternalInput")
    xt = nc.dram_tensor("xt", [C, nloc], BF16, kind="ExternalInput")
    mc = nc.dram_tensor("mc", [C, nloc], BF16, kind="ExternalInput")
    wqt = nc.dram_tensor("wqt", [C, C], BF16, kind="ExternalInput")
    wkt = nc.dram_tensor("wkt", [C, C], BF16, kind="ExternalInput")
    wvt = nc.dram_tensor("wvt", [C, C], BF16, kind="ExternalInput")
    hrep = nc.dram_tensor("hrep", [C, C], BF16, kind="ExternalInput")
    wost = nc.dram_tensor("wost", [C, C], BF16, kind="ExternalInput")
    wo3t = nc.dram_tensor("wo3t", [C, C], BF16, kind="ExternalInput")
    idt = nc.dram_tensor("idt", [C, C], BF16, kind="ExternalInput")
    out = nc.dram_tensor("out", [C, nloc], F32, kind="ExternalOutput")

    with tile.TileContext(nc) as tc:
        with tc.tile_pool(name="wts", bufs=1) as wpool, \
             tc.tile_pool(name="xin", bufs=1) as xpool, \
             tc.tile_pool(name="outp", bufs=1) as outp, \
             tc.tile_pool(name="etp", bufs=3) as etp, \
             tc.tile_pool(name="ktp", bufs=2) as ktp, \
             tc.tile_pool(name="vtp", bufs=2) as vtp, \
             tc.tile_pool(name="ep", bufs=2) as ep, \
             tc.tile_pool(name="uvp", bufs=2) as uvp, \
             tc.tile_pool(name="prp", bufs=3) as prp, \
             tc.tile_pool(name="scrp", bufs=2) as scrp, \
             tc.tile_pool(name="smp", bufs=2) as smp, \
             tc.tile_pool(name="pkt", bufs=2, space="PSUM") as pkt, \
             tc.tile_pool(name="pvt", bufs=1, space="PSUM") as pvt, \
             tc.tile_pool(name="pst", bufs=1, space="PSUM") as pst, \
             tc.tile_pool(name="pac", bufs=2, space="PSUM") as pac:

            w_q = wpool.tile([C, C], BF16, tag="wq")
            w_k = wpool.tile([C, C], BF16, tag="wk")
            w_v = wpool.tile([C, C], BF16, tag="wv")
            w_h = wpool.tile([C, C], BF16, tag="wh")
            w_os = wpool.tile([C, C], BF16, tag="wos")
            w_o3 = wpool.tile([C, C], BF16, tag="wo3")
            w_id = wpool.tile([C, C], BF16, tag="wid")
            nc.sync.dma_start(w_q[:], wqt[:])
            nc.sync.dma_start(w_k[:], wkt[:])
            nc.sync.dma_start(w_v[:], wvt[:])
            nc.sync.dma_start(w_h[:], hrep[:])
            nc.sync.dma_start(w_os[:], wost[:])
            nc.sync.dma_start(w_o3[:], wo3t[:])
            nc.sync.dma_start(w_id[:], idt[:])

            xt_sb = xpool.tile([C, nloc], BF16, tag="xt")
            nc.sync.dma_start(xt_sb[:], xt[:])
            mc_sb = xpool.tile([C, nloc], BF16, tag="mc")
            nc.sync.dma_start(mc_sb[:], mc[:])

            out_sb = outp.tile([C, nloc], F32, tag="osb")

            col_off = 0
            node_off = 0
            for (Kb, CH, nchunks) in spec:
                ccols = Kb * CH
                pieces = _pieces(Kb, CH)
                for chi in range(nchunks):
                    n0 = node_off + chi * CH

                    et_t = etp.tile([C, maxc], BF16, tag="et")
                    nc.sync.dma_start(et_t[:, :ccols],
                                      et[:, col_off:col_off + ccols])

                    acc = pac.tile([C, 512], F32, tag="acc")
                    z_ps = acc[:, 0:CH]
                    us_ps = acc[:, 128:128 + CH]
                    o_ps = acc[:, 256:256 + CH]
                    q_ps = acc[:, 384:384 + CH]

                    nc.tensor.matmul(q_ps, w_q[:], xt_sb[:, n0:n0 + CH],
                                     start=True, stop=True)
                    q_sb = smp.tile([C, 128], BF16, tag="qsb")
                    nc.scalar.copy(q_sb[:, :CH], q_ps)

                    vt_t = vtp.tile([C, maxc], BF16, tag="vt")
                    e_t = ep.tile([C, maxc], BF16, tag="e")
                    uv_t = uvp.tile([C, maxc], BF16, tag="uv")

                    # process pieces in pairs: matmuls at <=512 (PSUM bank),
                    # exp / VT-evac at <=1024 (halves ACT per-inst taxes)
                    groups = [pieces[i:i + 2] for i in range(0, len(pieces), 2)]
                    for gi, grp in enumerate(groups):
                        g0 = grp[0][0]
                        g1 = grp[-1][1]
                        gc = (g1 - g0) * CH
                        gsl = slice(g0 * CH, g1 * CH)
                        vps = pvt.tile([C, 1024], F32, tag="vtp")
                        sps = pst.tile([C, 1024], F32, tag="sp")
                        off = 0
                        for (j0, j1) in grp:
                            nj = j1 - j0
                            pc = nj * CH
                            kps = pkt.tile([C, 512], F32, tag="ktp")
                            nc.tensor.matmul(kps[:, :pc], w_k[:],
                                             et_t[:, j0 * CH:j1 * CH],
                                             start=True, stop=True)
                            nc.tensor.matmul(vps[:, off:off + pc], w_v[:],
                                             et_t[:, j0 * CH:j1 * CH],
                                             start=True, stop=True)
                            # prod = KT * q (q broadcast over j; KT read
                            # from PSUM at 1x -- beats an extra evacuation)
                            pr = prp.tile([C, 512], BF16, tag="prod")
                            qb = q_sb[:, :CH].unsqueeze(1).broadcast_to(
                                (C, nj, CH))
                            nc.vector.tensor_mul(
                                pr[:, :pc].rearrange("p (j n) -> p j n",
                                                     n=CH),
                                kps[:, :pc].rearrange("p (j n) -> p j n",
                                                      n=CH),
                                qb)
                            # s_rep = Hrep @ prod (head sum, replicated)
                            nc.tensor.matmul(sps[:, off:off + pc], w_h[:],
                                             pr[:, :pc],
                                             start=True, stop=True)
                            off += pc
                        # e = exp(s)
                        nc.scalar.activation(e_t[:, gsl], sps[:, :gc],
                                             ACTF.Exp)
                        # VT evac (ACT) -> bf16 SBUF so uv runs at 2x
                        nc.scalar.copy(vt_t[:, gsl], vps[:, :gc])
                        # uv = e * v (split DVE/GpSimd for engine balance)
                        if gi % 3 == 2:
                            nc.gpsimd.tensor_mul(uv_t[:, gsl], e_t[:, gsl],
                                                 vt_t[:, gsl])
                        else:
                            nc.vector.tensor_mul(uv_t[:, gsl], e_t[:, gsl],
                                                 vt_t[:, gsl])

                    # z = sum_j e ; usum = sum_j uv  (identity matmuls, PE)
                    for j in range(Kb):
                        nc.tensor.matmul(z_ps, w_id[:],
                                         e_t[:, j * CH:(j + 1) * CH],
                                         start=(j == 0), stop=(j == Kb - 1))
                    for j in range(Kb):
                        nc.tensor.matmul(us_ps, w_id[:],
                                         uv_t[:, j * CH:(j + 1) * CH],
                                         start=(j == 0), stop=(j == Kb - 1))

                    # umax = max_j uv  (pairwise tree on DVE, bf16 2x)
                    scr = scrp.tile([C, maxh], BF16, tag="scr")
                    jj = Kb
                    h = jj // 2
                    nc.vector.tensor_max(scr[:, :h * CH], uv_t[:, :h * CH],
                                         uv_t[:, h * CH:2 * h * CH])
                    if jj % 2:
                        nc.vector.tensor_max(
                            scr[:, :CH], scr[:, :CH],
                            uv_t[:, 2 * h * CH:(2 * h + 1) * CH])
                    jj = h
                    while jj > 1:
                        h = jj // 2
                        nc.vector.tensor_max(scr[:, :h * CH], scr[:, :h * CH],
                                             scr[:, h * CH:2 * h * CH])
                        if jj % 2:
                            nc.vector.tensor_max(
                                scr[:, :CH], scr[:, :CH],
                                scr[:, 2 * h * CH:(2 * h + 1) * CH])
                        jj = h

                    # epilogue: rz = 1/max(z - mcorr, eps); scale; project
                    zc = smp.tile([C, 128], F32, tag="zc")
                    nc.vector.scalar_tensor_tensor(
                        zc[:, :CH], z_ps, 0.0, mc_sb[:, n0:n0 + CH],
                        op0=ALU.bypass, op1=ALU.subtract)
                    nc.vector.tensor_scalar_max(zc[:, :CH], zc[:, :CH], 1e-20)
                    rz = smp.tile([C, 128], F32, tag="rz")
                    nc.vector.reciprocal(rz[:, :CH], zc[:, :CH])
                    wsn = smp.tile([C, 128], BF16, tag="wsn")
                    nc.vector.tensor_mul(wsn[:, :CH], us_ps, rz[:, :CH])
                    mxn = smp.tile([C, 128], BF16, tag="mxn")
                    nc.gpsimd.tensor_mul(mxn[:, :CH], scr[:, :CH], rz[:, :CH])

                    nc.tensor.matmul(o_ps, w_os[:], wsn[:, :CH],
                                     start=True, stop=False)
                    nc.tensor.matmul(o_ps, w_o3[:], mxn[:, :CH],
                                     start=False, stop=True)
                    nc.scalar.copy(out_sb[:, n0:n0 + CH], o_ps)

                    col_off += ccols
                node_off += nchunks * CH

            nc.sync.dma_start(out[:], out_sb[:])

    nc.compile()
    _NC_CACHE[spec] = nc
    return nc


def build_nc(spec):
    return _build_nc(spec)


def _perm_dh(w):
    """torch Linear weight [(h*32+d), cin] -> stationary [cin, (4d+h)]."""
    wt = np.asarray(w).reshape(H, D, -1)
    return np.ascontiguousarray(np.transpose(wt, (2, 1, 0)).reshape(-1, H * D))


def _choose_ch(maxcount, Kb):
    """Pick chunk node-count CH: balance pad waste vs per-chunk overhead."""
    best = None
    for CH in (128, 64, 32, 16):
        nch = max(1, -(-maxcount // CH))
        pad_cols = (nch * CH - maxcount) * Kb
        cost = pad_cols + 700 * nch
        if best is None or cost < best[0]:
            best = (cost, CH, nch)
    return best[1], best[2]


def prep_inputs(h_X, h_E, mask_attn, W_Q, W_K, W_V, W_O):
    h_X = np.asarray(h_X, dtype=np.float32)
    h_E = np.asarray(h_E, dtype=np.float32)
    mask_attn = np.asarray(mask_attn)
    W_Q = np.asarray(W_Q, dtype=np.float32)
    W_K = np.asarray(W_K, dtype=np.float32)
    W_V = np.asarray(W_V, dtype=np.float32)
    W_O = np.asarray(W_O, dtype=np.float32)

    B, N, Kn, Cin = h_E.shape
    BN = B * N

    mask = mask_attn.reshape(BN, Kn) > 0
    cnt = mask.sum(axis=1).astype(np.int64)

    # compact E: active neighbors first, zero padding after
    A = np.zeros((BN, Kn, Cin), dtype=NPBF16)
    nz_node, nz_j = np.nonzero(mask)
    cum = np.zeros(BN + 1, dtype=np.int64)
    np.cumsum(cnt, out=cum[1:])
    pos = np.arange(len(nz_node)) - cum[nz_node]
    A[nz_node, pos] = h_E.reshape(BN, Kn, Cin)[nz_node, nz_j]

    # bucket assignment
    bks = np.array(BUCKET_KS)
    bid = np.searchsorted(bks, cnt)           # smallest bucket >= cnt

    # per (bucket, core) node lists, equalized + chunk-padded across cores
    spec = []
    core_slots = [[] for _ in range(NCORES)]
    for bi, Kb in enumerate(BUCKET_KS):
        nodes_b = np.nonzero(bid == bi)[0]
        if len(nodes_b) == 0:
            continue
        per_core = [nodes_b[c::NCORES] for c in range(NCORES)]
        maxcount = max(len(p) for p in per_core)
        CH, nch = _choose_ch(maxcount, Kb)
        npad = nch * CH
        spec.append((Kb, CH, nch))
        for c in range(NCORES):
            ids = np.full(npad, -1, dtype=np.int64)
            ids[:len(per_core[c])] = per_core[c]
            core_slots[c].append((Kb, CH, nch, ids))
    spec = tuple(spec)

    nloc = sum(ch * nch for (_, ch, nch) in spec)

    xf = h_X.reshape(BN, C)
    wqt = _perm_dh(W_Q / np.sqrt(D)).astype(NPBF16)
    wkt = _perm_dh(W_K).astype(NPBF16)
    wvt = _perm_dh(W_V).astype(NPBF16)
    idx = np.arange(C)
    hrep = (idx[:, None] % H == idx[None, :] % H).astype(NPBF16)
    ident = np.eye(C, dtype=NPBF16)
    wos = W_O[:, :C] + W_O[:, C:2 * C]
    wo3 = W_O[:, 2 * C:]
    wost = np.ascontiguousarray(
        wos.T.reshape(H, D, C).transpose(1, 0, 2).reshape(C, C)).astype(NPBF16)
    wo3t = np.ascontiguousarray(
        wo3.T.reshape(H, D, C).transpose(1, 0, 2).reshape(C, C)).astype(NPBF16)

    in_maps = []
    slot_list = []
    for c in range(NCORES):
        slots = np.concatenate([ids for (_, _, _, ids) in core_slots[c]])
        slot_list.append(slots)
        valid = slots >= 0
        sc = np.where(valid, slots, 0)

        xg = xf[sc]
        xg[~valid] = 0.0
        xtc = np.ascontiguousarray(xg.T).astype(NPBF16)

        mcv = np.zeros(len(slots), dtype=np.float32)
        off = 0
        etc = np.empty((C, sum(kb * ch * nch
                               for (kb, ch, nch, _) in core_slots[c])),
                       dtype=NPBF16)
        coff = 0
        for (Kb, CH, nch, ids) in core_slots[c]:
            v = ids >= 0
            mcv[off:off + len(ids)] = np.where(v, Kb - cnt[np.where(v, ids, 0)],
                                               Kb)
            for chi in range(nch):
                cid = ids[chi * CH:(chi + 1) * CH]
                cv = cid >= 0
                blk = A[np.where(cv, cid, 0), :Kb, :]      # [CH, Kb, C]
                blk[~cv] = 0
                etc[:, coff:coff + Kb * CH] = (
                    blk.transpose(2, 1, 0).reshape(C, Kb * CH))
                coff += Kb * CH
            off += len(ids)
        mcc = np.ascontiguousarray(
            np.broadcast_to(mcv, (C, len(slots)))).astype(NPBF16)

        in_maps.append({
            "et": etc, "xt": xtc, "mc": mcc,
            "wqt": wqt, "wkt": wkt, "wvt": wvt, "hrep": hrep,
            "wost": wost, "wo3t": wo3t, "idt": ident,
        })

    prep_inputs._slots = slot_list
    prep_inputs._spec = spec
    prep_inputs._shape = (B, N)
    return in_maps, spec


def assemble_output(results, B, N):
    BN = B * N
    outf = np.zeros((BN, C), np.float32)
    for i, r in enumerate(results):
        slots = prep_inputs._slots[i]
        valid = slots >= 0
        outf[slots[valid]] = r["out"].T[valid]
    return outf.reshape(B, N, C)


def kernel(h_X, h_E, mask_attn, W_Q, W_K, W_V, W_O):
    B, N = np.asarray(h_X).shape[:2]
    in_maps, spec = prep_inputs(h_X, h_E, mask_attn, W_Q, W_K, W_V, W_O)
    nc = _build_nc(spec)
    res = run_bass_kernel_spmd(nc, in_maps, core_ids=list(range(NCORES)))
    return assemble_output(res.results, B, N)
